# revision 4
# baseline (speedup 1.0000x reference)
"""Trainium2 Bass kernel for a GRU-style recurrent cell.

Strategy
--------
8-way data parallel over the batch (8 rows/core, weights replicated).
Per core, the sequential scan is replaced by a quasi-DEER fixed-point
iteration: the diagonal gating recurrence h_t = a_t*h_{t-1} + b_t is
solved exactly each sweep with the DVE tensor_tensor_scan, while the
dense U-matmul couplings are iterated (Jacobi/Gauss-Seidel hybrid).
Empirically this converges to ~3e-5 relative error in 7 sweeps per
chunk, chunk-length independent, so the whole recurrence costs ~7
batched GEMM passes instead of 2048 tiny sequential matmuls (which are
LDWEIGHTS-bound on the PE).

Phases per 512-step chunk:
  1) precompute A_g = W_gx @ x + b_g for the 3 gates (fp16, spilled to DRAM)
  2) SWEEPS fixed-point sweeps; each sweep streams A back, does the
     3 U-GEMMs + activations + the chained scans per 64-step subchunk
  3) outs = Wb @ h + bb for the converged chunk trajectory
"""

import os
import numpy as np

T_FULL = 2048
B_CORE = 8          # batch rows per core
H = 512             # hidden dim
I3 = 768            # 3*I_DIM, x-part of the cell input
BD = 256            # output dim of Wb
N_CORES = 8
SUB = 64            # timesteps per subchunk (512 psum cols = 64*8)
NCOL = SUB * B_CORE # 512 columns per matmul tile

_CACHE = {}


def _build(T, sweeps):
    import concourse.bass as bass
    import concourse.tile as tile
    from concourse import bacc, mybir

    f16 = mybir.dt.float16
    f32 = mybir.dt.float32
    AF = mybir.ActivationFunctionType
    OP = mybir.AluOpType

    L = min(512, T)          # chunk length
    NCH = T // L             # chunks
    NSUB = L // SUB          # subchunks per chunk

    nc = bacc.Bacc("TRN2", target_bir_lowering=False, debug=False,
                   num_devices=N_CORES)

    # ---- DRAM I/O (per core) ----
    xsT = nc.dram_tensor("xsT", [I3, B_CORE * T], f16, kind="ExternalInput").ap()
    wxT = {g: nc.dram_tensor(f"wx{g}T", [I3, H], f16, kind="ExternalInput").ap()
           for g in "zrh"}
    uT = {g: nc.dram_tensor(f"u{g}T", [H, H], f16, kind="ExternalInput").ap()
          for g in "zrh"}
    wbT = nc.dram_tensor("wbT", [H, BD], f16, kind="ExternalInput").ap()
    bias = {g: nc.dram_tensor(f"b{g}", [128, 4], f32, kind="ExternalInput").ap()
            for g in "zrh"}
    bbv = nc.dram_tensor("bb", [128, 2], f32, kind="ExternalInput").ap()
    ident_d = nc.dram_tensor("ident", [128, 128], f16, kind="ExternalInput").ap()

    outs_d = nc.dram_tensor("outs", [BD, B_CORE * T], f32, kind="ExternalOutput").ap()
    hfin_d = nc.dram_tensor("hfin", [128, 4 * B_CORE], f32, kind="ExternalOutput").ap()

    with tile.TileContext(nc) as tc:
        with (
            tc.tile_pool(name="dram", bufs=1, space="DRAM") as dpool,
            tc.tile_pool(name="wpool", bufs=1) as wpool,
            tc.tile_pool(name="hpool", bufs=1) as hpool,
            tc.tile_pool(name="xs", bufs=12) as xspool,
            tc.tile_pool(name="aload", bufs=26) as apool,
            tc.tile_pool(name="ev", bufs=6) as evpool,
            tc.tile_pool(name="abuf", bufs=10) as abpool,
            tc.tile_pool(name="bbuf", bufs=10) as bbpool,
            tc.tile_pool(name="tmp", bufs=6) as tpool,
            tc.tile_pool(name="oev", bufs=3) as oevpool,
            tc.tile_pool(name="psum", bufs=8, space="PSUM") as pspool,
        ):
            # A_g spill tensors in DRAM, [512, B*T] cols = b*T + t
            A_d = {g: dpool.tile([H, B_CORE * T], f16, tag=f"A{g}", name=f"A{g}") for g in "zrh"}

            # ---- load constant weights into SBUF ----
            wx_t = {}   # per gate: 6 tiles [128, 512] (k-tile rows)
            u_t = {}    # per gate: 4 tiles [128, 512]
            for g in "zrh":
                wx_t[g] = []
                for j in range(6):
                    w = wpool.tile([128, H], f16, tag=f"wx{g}{j}")
                    nc.sync.dma_start(w[:], wxT[g][j * 128:(j + 1) * 128, :])
                    wx_t[g].append(w)
                u_t[g] = []
                for j in range(4):
                    w = wpool.tile([128, H], f16, tag=f"u{g}{j}")
                    nc.sync.dma_start(w[:], uT[g][j * 128:(j + 1) * 128, :])
                    u_t[g].append(w)
            wb_t = []
            for j in range(4):
                w = wpool.tile([128, BD], f16, tag=f"wb{j}")
                nc.sync.dma_start(w[:], wbT[j * 128:(j + 1) * 128, :])
                wb_t.append(w)
            ident = wpool.tile([128, 128], f16, tag="ident")
            nc.sync.dma_start(ident[:], ident_d[:, :])
            bias_t = {}
            for g in "zrh":
                bt = wpool.tile([128, 4], f32, tag=f"bias{g}")
                nc.sync.dma_start(bt[:], bias[g][:, :])
                bias_t[g] = bt
            bb_t = wpool.tile([128, 2], f32, tag="biasb")
            nc.sync.dma_start(bb_t[:], bbv[:, :])

            # persistent h trajectory tiles: hT[j][n] = [128, 8*(SUB+1)]
            # col b*(SUB+1) + k ; k=0 boundary (h at step n*SUB-1), k=1..SUB
            # h at steps n*SUB .. n*SUB+SUB-1 (local to chunk)
            KW = SUB + 1
            hT = [[hpool.tile([128, B_CORE * KW], f16, tag=f"h{j}_{n}", name=f"h{j}_{n}")
                   for n in range(NSUB)] for j in range(4)]
            hb = [hpool.tile([128, B_CORE], f16, tag=f"hb{j}", name=f"hb{j}") for j in range(4)]
            for j in range(4):
                nc.vector.memset(hb[j][:], 0.0)

            def h_prev_ap(j, n):
                # [128, 8, SUB] free dims (b, t): h_{t-1} for steps in subchunk n
                return hT[j][n][:].rearrange("p (b k) -> p b k", b=B_CORE)[:, :, 0:SUB]

            def h_cur_ap(j, n):
                return hT[j][n][:].rearrange("p (b k) -> p b k", b=B_CORE)[:, :, 1:KW]

            for c in range(NCH):
                c0 = c * L

                # ---- phase 1: A_g for this chunk ----
                for n in range(NSUB):
                    t0 = c0 + n * SUB
                    xs_tiles = []
                    for j in range(6):
                        xt = xspool.tile([128, NCOL], f16, tag="xs")
                        src = xsT[j * 128:(j + 1) * 128, :] \
                            .rearrange("p (b t) -> p b t", b=B_CORE)[:, :, t0:t0 + SUB]
                        nc.sync.dma_start(xt[:].rearrange("p (b t) -> p b t", b=B_CORE), src)
                        xs_tiles.append(xt)
                    for g in "zrh":
                        for i in range(4):
                            ps = pspool.tile([128, NCOL], f32, tag="ps")
                            for j in range(6):
                                nc.tensor.matmul(
                                    ps[:], wx_t[g][j][:, i * 128:(i + 1) * 128],
                                    xs_tiles[j][:], start=(j == 0), stop=(j == 5))
                            ev = evpool.tile([128, NCOL], f16, tag="ev")
                            nc.scalar.activation(ev[:], ps[:], AF.Identity,
                                                 bias=bias_t[g][:, i:i + 1])
                            dst = A_d[g][i * 128:(i + 1) * 128, :] \
                                .rearrange("p (b t) -> p b t", b=B_CORE)[:, :, t0:t0 + SUB]
                            nc.gpsimd.dma_start(
                                dst, ev[:].rearrange("p (b t) -> p b t", b=B_CORE))

                # ---- init h trajectory ----
                if c == 0:
                    for j in range(4):
                        for n in range(NSUB):
                            nc.vector.memset(hT[j][n][:], 0.0)
                else:
                    # stale trajectory from previous chunk is a fine init;
                    # only the chunk boundary column must be refreshed
                    for j in range(4):
                        nc.vector.tensor_copy(hT[j][0][:, 0::KW], hb[j][:])

                # ---- phase 2: sweeps ----
                for s in range(sweeps):
                    scan_q = []
                    for n in range(NSUB):
                        t0 = c0 + n * SUB
                        # A tiles for this subchunk
                        At = {}
                        for g in "zrh":
                            At[g] = []
                            for i in range(4):
                                at = apool.tile([128, NCOL], f16, tag="at")
                                src = A_d[g][i * 128:(i + 1) * 128, :] \
                                    .rearrange("p (b t) -> p b t", b=B_CORE)[:, :, t0:t0 + SUB]
                                nc.sync.dma_start(
                                    at[:].rearrange("p (b t) -> p b t", b=B_CORE), src)
                                At[g].append(at)
                        a_list, rh_list = [], []
                        for i in range(4):
                            # update gate -> a = sigma(-pre_z) = 1 - z
                            pz = pspool.tile([128, NCOL], f32, tag="ps")
                            for j in range(4):
                                nc.tensor.matmul(pz[:], u_t["z"][j][:, i * 128:(i + 1) * 128],
                                                 h_prev_ap(j, n), start=(j == 0), stop=False)
                            nc.tensor.matmul(pz[:], ident[:], At["z"][i][:],
                                             start=False, stop=True)
                            a_t = abpool.tile([128, NCOL], f16, tag="a")
                            nc.scalar.activation(a_t[:], pz[:], AF.Sigmoid, scale=-1.0)
                            a_list.append(a_t)
                            # reset gate
                            pr = pspool.tile([128, NCOL], f32, tag="ps")
                            for j in range(4):
                                nc.tensor.matmul(pr[:], u_t["r"][j][:, i * 128:(i + 1) * 128],
                                                 h_prev_ap(j, n), start=(j == 0), stop=False)
                            nc.tensor.matmul(pr[:], ident[:], At["r"][i][:],
                                             start=False, stop=True)
                            r_t = tpool.tile([128, NCOL], f16, tag="rt")
                            nc.scalar.activation(r_t[:], pr[:], AF.Sigmoid)
                            rh = tpool.tile([128, NCOL], f16, tag="rh")
                            nc.vector.tensor_mul(
                                rh[:].rearrange("p (b t) -> p b t", b=B_CORE),
                                r_t[:].rearrange("p (b t) -> p b t", b=B_CORE),
                                h_prev_ap(i, n))
                            rh_list.append(rh)
                        bneg_list = []
                        for i in range(4):
                            ph = pspool.tile([128, NCOL], f32, tag="ps")
                            for j in range(4):
                                nc.tensor.matmul(ph[:], u_t["h"][j][:, i * 128:(i + 1) * 128],
                                                 rh_list[j][:], start=(j == 0), stop=False)
                            nc.tensor.matmul(ph[:], ident[:], At["h"][i][:],
                                             start=False, stop=True)
                            ht_t = tpool.tile([128, NCOL], f16, tag="ht")
                            nc.scalar.activation(ht_t[:], ph[:], AF.Tanh)
                            # bneg = (a-1)*h_tilde = -z*h_tilde
                            bn = bbpool.tile([128, NCOL], f16, tag="bn")
                            nc.vector.scalar_tensor_tensor(
                                bn[:], a_list[i][:], 1.0, ht_t[:],
                                OP.subtract, OP.mult)
                            bneg_list.append(bn)
                        scan_q.append((n, a_list, bneg_list))
                        # lagged scans: emit scans for subchunk n-1
                        if len(scan_q) >= 2:
                            _emit_scan(nc, OP, hT, scan_q.pop(0), B_CORE, SUB, KW, NSUB)
                    while scan_q:
                        _emit_scan(nc, OP, hT, scan_q.pop(0), B_CORE, SUB, KW, NSUB)

                # ---- phase 3: outs for this chunk ----
                for n in range(NSUB):
                    t0 = c0 + n * SUB
                    for m in range(2):
                        po = pspool.tile([128, NCOL], f32, tag="ps")
                        for j in range(4):
                            nc.tensor.matmul(po[:], wb_t[j][:, m * 128:(m + 1) * 128],
                                             h_cur_ap(j, n), start=(j == 0), stop=(j == 3))
                        oe = oevpool.tile([128, NCOL], f32, tag="oe")
                        nc.scalar.activation(oe[:], po[:], AF.Identity,
                                             bias=bb_t[:, m:m + 1])
                        dst = outs_d[m * 128:(m + 1) * 128, :] \
                            .rearrange("p (b t) -> p b t", b=B_CORE)[:, :, t0:t0 + SUB]
                        nc.gpsimd.dma_start(
                            dst, oe[:].rearrange("p (b t) -> p b t", b=B_CORE))

                # ---- carry boundary to next chunk ----
                for j in range(4):
                    nc.vector.tensor_copy(hb[j][:], hT[j][NSUB - 1][:, SUB::KW])

            # ---- final hidden state ----
            hf = oevpool.tile([128, 4 * B_CORE], f32, tag="hf")
            for j in range(4):
                nc.vector.tensor_copy(hf[:, j * B_CORE:(j + 1) * B_CORE], hb[j][:])
            nc.sync.dma_start(hfin_d[:, :], hf[:])

    nc.compile()
    return nc


def _emit_scan(nc, OP, hT, item, B, SUB, KW, NSUB):
    n, a_list, bneg_list = item
    for j in range(4):
        ht = hT[j][n][:]
        at = a_list[j][:]
        bt = bneg_list[j][:]
        for b in range(B):
            # state = a*state - bneg  (bneg = -z*h_tilde)
            nc.vector.tensor_tensor_scan(
                ht[:, b * KW + 1:(b + 1) * KW],
                at[:, b * SUB:(b + 1) * SUB],
                bt[:, b * SUB:(b + 1) * SUB],
                ht[:, b * KW:b * KW + 1], OP.mult, OP.subtract)
        # propagate boundary into next subchunk tile (col SUB -> col 0)
        if n + 1 < NSUB:
            nc.vector.tensor_copy(hT[j][n + 1][:, 0::KW], hT[j][n][:, SUB::KW])


def _get_nc(T, sweeps):
    key = (T, sweeps)
    if key not in _CACHE:
        _CACHE[key] = _build(T, sweeps)
    return _CACHE[key]


def kernel(x, Wz, bz, Wr, br, Wh, bh, Wb, bb):
    from concourse.bass_utils import run_bass_kernel_spmd

    x = np.asarray(x)
    B_full, _, T, I = x.shape
    sweeps = int(os.environ.get("DEER_SWEEPS", "7"))
    nc = _get_nc(T, sweeps)

    f16 = np.float16
    Wz, Wr, Wh, Wb = (np.asarray(a, np.float32) for a in (Wz, Wr, Wh, Wb))
    bz, br, bh, bb = (np.asarray(a, np.float32) for a in (bz, br, bh, bb))

    common = {}
    for g, W in (("z", Wz), ("r", Wr), ("h", Wh)):
        common[f"wx{g}T"] = np.ascontiguousarray(W[:, :I3].T, f16)
        common[f"u{g}T"] = np.ascontiguousarray(W[:, I3:].T, f16)
    common["wbT"] = np.ascontiguousarray(Wb.T, f16)
    for g, b in (("z", bz), ("r", br), ("h", bh)):
        common[f"b{g}"] = np.ascontiguousarray(b.reshape(4, 128).T, np.float32)
    common["bb"] = np.ascontiguousarray(bb.reshape(2, 128).T, np.float32)
    common["ident"] = np.eye(128, dtype=f16)

    in_maps = []
    for c in range(N_CORES):
        xc = x[c * B_CORE:(c + 1) * B_CORE]          # [8, 3, T, I]
        # xsT [768, B*T] with col = b*T + t ; row = s*I + i
        xst = np.ascontiguousarray(
            xc.transpose(1, 3, 0, 2).reshape(I3, B_CORE * T), f16)
        m = dict(common)
        m["xsT"] = xst
        in_maps.append(m)

    res = run_bass_kernel_spmd(nc, in_maps, core_ids=list(range(N_CORES)))

    outs = np.empty((B_full, T, BD), np.float32)
    hfin = np.empty((B_full, H), np.float32)
    for c in range(N_CORES):
        o = res.results[c]["outs"].reshape(BD, B_CORE, T)
        outs[c * B_CORE:(c + 1) * B_CORE] = o.transpose(1, 2, 0)
        hf = res.results[c]["hfin"].reshape(128, 4, B_CORE)
        hfin[c * B_CORE:(c + 1) * B_CORE] = hf.transpose(2, 1, 0).reshape(B_CORE, H)
    return outs, hfin


# revision 5
# speedup vs baseline: 1.0090x; 1.0090x over previous
"""Trainium2 Bass kernel for a GRU-style recurrent cell.

Strategy
--------
8-way data parallel over the batch (8 rows/core, weights replicated).
Per core, the sequential scan is replaced by a quasi-DEER fixed-point
iteration: the diagonal gating recurrence h_t = a_t*h_{t-1} + b_t is
solved exactly each sweep with the DVE tensor_tensor_scan, while the
dense U-matmul couplings are iterated (Jacobi/Gauss-Seidel hybrid).
Empirically this converges to ~3e-5 relative error in 7 sweeps per
chunk, chunk-length independent, so the whole recurrence costs ~7
batched GEMM passes instead of 2048 tiny sequential matmuls (which are
LDWEIGHTS-bound on the PE).

Phases per 512-step chunk:
  1) precompute A_g = W_gx @ x + b_g for the 3 gates (fp16, spilled to DRAM)
  2) SWEEPS fixed-point sweeps; each sweep streams A back, does the
     3 U-GEMMs + activations + the chained scans per 64-step subchunk
  3) outs = Wb @ h + bb for the converged chunk trajectory
"""

import os
import numpy as np

T_FULL = 2048
B_CORE = 8          # batch rows per core
H = 512             # hidden dim
I3 = 768            # 3*I_DIM, x-part of the cell input
BD = 256            # output dim of Wb
N_CORES = 8
SUB = 64            # timesteps per subchunk (512 psum cols = 64*8)
NCOL = SUB * B_CORE # 512 columns per matmul tile

_CACHE = {}


def _build(T, sweeps):
    import concourse.bass as bass
    import concourse.tile as tile
    from concourse import bacc, mybir

    f16 = mybir.dt.float16
    f32 = mybir.dt.float32
    AF = mybir.ActivationFunctionType
    OP = mybir.AluOpType

    L = min(512, T)          # chunk length
    NCH = T // L             # chunks
    NSUB = L // SUB          # subchunks per chunk

    nc = bacc.Bacc("TRN2", target_bir_lowering=False, debug=False,
                   num_devices=N_CORES)

    # ---- DRAM I/O (per core) ----
    xsT = nc.dram_tensor("xsT", [I3, B_CORE * T], f16, kind="ExternalInput").ap()
    wxT = {g: nc.dram_tensor(f"wx{g}T", [I3, H], f16, kind="ExternalInput").ap()
           for g in "zrh"}
    uT = {g: nc.dram_tensor(f"u{g}T", [H, H], f16, kind="ExternalInput").ap()
          for g in "zrh"}
    wbT = nc.dram_tensor("wbT", [H, BD], f16, kind="ExternalInput").ap()
    bias = {g: nc.dram_tensor(f"b{g}", [128, 4], f32, kind="ExternalInput").ap()
            for g in "zrh"}
    bbv = nc.dram_tensor("bb", [128, 2], f32, kind="ExternalInput").ap()
    ident_d = nc.dram_tensor("ident", [128, 128], f16, kind="ExternalInput").ap()

    outs_d = nc.dram_tensor("outs", [BD, B_CORE * T], f32, kind="ExternalOutput").ap()
    hfin_d = nc.dram_tensor("hfin", [128, 4 * B_CORE], f32, kind="ExternalOutput").ap()

    with tile.TileContext(nc) as tc:
        with (
            tc.tile_pool(name="dram", bufs=1, space="DRAM") as dpool,
            tc.tile_pool(name="wpool", bufs=1) as wpool,
            tc.tile_pool(name="hpool", bufs=1) as hpool,
            tc.tile_pool(name="xs", bufs=12) as xspool,
            tc.tile_pool(name="aload", bufs=26) as apool,
            tc.tile_pool(name="ev", bufs=6) as evpool,
            tc.tile_pool(name="abuf", bufs=10) as abpool,
            tc.tile_pool(name="bbuf", bufs=10) as bbpool,
            tc.tile_pool(name="tmp", bufs=6) as tpool,
            tc.tile_pool(name="oev", bufs=3) as oevpool,
            tc.tile_pool(name="psum", bufs=8, space="PSUM") as pspool,
        ):
            # A_g spill tensors in DRAM, [512, B*T] cols = b*T + t
            A_d = {g: dpool.tile([H, B_CORE * T], f16, tag=f"A{g}", name=f"A{g}") for g in "zrh"}

            # ---- load constant weights into SBUF ----
            wx_t = {}   # per gate: 6 tiles [128, 512] (k-tile rows)
            u_t = {}    # per gate: 4 tiles [128, 512]
            for g in "zrh":
                wx_t[g] = []
                for j in range(6):
                    w = wpool.tile([128, H], f16, tag=f"wx{g}{j}")
                    nc.sync.dma_start(w[:], wxT[g][j * 128:(j + 1) * 128, :])
                    wx_t[g].append(w)
                u_t[g] = []
                for j in range(4):
                    w = wpool.tile([128, H], f16, tag=f"u{g}{j}")
                    nc.sync.dma_start(w[:], uT[g][j * 128:(j + 1) * 128, :])
                    u_t[g].append(w)
            wb_t = []
            for j in range(4):
                w = wpool.tile([128, BD], f16, tag=f"wb{j}")
                nc.sync.dma_start(w[:], wbT[j * 128:(j + 1) * 128, :])
                wb_t.append(w)
            ident = wpool.tile([128, 128], f16, tag="ident")
            nc.sync.dma_start(ident[:], ident_d[:, :])
            bias_t = {}
            for g in "zrh":
                bt = wpool.tile([128, 4], f32, tag=f"bias{g}")
                nc.sync.dma_start(bt[:], bias[g][:, :])
                bias_t[g] = bt
            bb_t = wpool.tile([128, 2], f32, tag="biasb")
            nc.sync.dma_start(bb_t[:], bbv[:, :])

            # persistent h trajectory tiles: hT[j][n] = [128, 8*(SUB+1)]
            # col b*(SUB+1) + k ; k=0 boundary (h at step n*SUB-1), k=1..SUB
            # h at steps n*SUB .. n*SUB+SUB-1 (local to chunk)
            KW = SUB + 1
            hT = [[hpool.tile([128, B_CORE * KW], f16, tag=f"h{j}_{n}", name=f"h{j}_{n}")
                   for n in range(NSUB)] for j in range(4)]
            hb = [hpool.tile([128, B_CORE], f16, tag=f"hb{j}", name=f"hb{j}") for j in range(4)]
            for j in range(4):
                nc.vector.memset(hb[j][:], 0.0)

            def h_prev_ap(j, n):
                # [128, 8, SUB] free dims (b, t): h_{t-1} for steps in subchunk n
                return hT[j][n][:].rearrange("p (b k) -> p b k", b=B_CORE)[:, :, 0:SUB]

            def h_cur_ap(j, n):
                return hT[j][n][:].rearrange("p (b k) -> p b k", b=B_CORE)[:, :, 1:KW]

            for c in range(NCH):
                c0 = c * L

                # ---- phase 1: A_g for this chunk ----
                for n in range(NSUB):
                    t0 = c0 + n * SUB
                    xs_tiles = []
                    for j in range(6):
                        xt = xspool.tile([128, NCOL], f16, tag="xs")
                        src = xsT[j * 128:(j + 1) * 128, :] \
                            .rearrange("p (b t) -> p b t", b=B_CORE)[:, :, t0:t0 + SUB]
                        nc.sync.dma_start(xt[:].rearrange("p (b t) -> p b t", b=B_CORE), src)
                        xs_tiles.append(xt)
                    for g in "zrh":
                        for i in range(4):
                            ps = pspool.tile([128, NCOL], f32, tag="ps")
                            for j in range(6):
                                nc.tensor.matmul(
                                    ps[:], wx_t[g][j][:, i * 128:(i + 1) * 128],
                                    xs_tiles[j][:], start=(j == 0), stop=(j == 5))
                            ev = evpool.tile([128, NCOL], f16, tag="ev")
                            nc.scalar.activation(ev[:], ps[:], AF.Identity,
                                                 bias=bias_t[g][:, i:i + 1])
                            dst = A_d[g][i * 128:(i + 1) * 128, :] \
                                .rearrange("p (b t) -> p b t", b=B_CORE)[:, :, t0:t0 + SUB]
                            nc.gpsimd.dma_start(
                                dst, ev[:].rearrange("p (b t) -> p b t", b=B_CORE))

                # ---- init h trajectory ----
                if c == 0:
                    for j in range(4):
                        for n in range(NSUB):
                            nc.vector.memset(hT[j][n][:], 0.0)
                else:
                    # stale trajectory from previous chunk is a fine init;
                    # only the chunk boundary column must be refreshed
                    for j in range(4):
                        nc.vector.tensor_copy(hT[j][0][:, 0::KW], hb[j][:])

                # ---- phase 2: sweeps ----
                for s in range(sweeps):
                    scan_q = []
                    for n in range(NSUB):
                        t0 = c0 + n * SUB
                        # A tiles for this subchunk
                        At = {}
                        for g in "zrh":
                            At[g] = []
                            for i in range(4):
                                at = apool.tile([128, NCOL], f16, tag="at")
                                src = A_d[g][i * 128:(i + 1) * 128, :] \
                                    .rearrange("p (b t) -> p b t", b=B_CORE)[:, :, t0:t0 + SUB]
                                nc.sync.dma_start(
                                    at[:].rearrange("p (b t) -> p b t", b=B_CORE), src)
                                At[g].append(at)
                        a_list, rh_list = [], []
                        for i in range(4):
                            # update gate -> a = sigma(-pre_z) = 1 - z
                            pz = pspool.tile([128, NCOL], f32, tag="ps")
                            for j in range(4):
                                nc.tensor.matmul(pz[:], u_t["z"][j][:, i * 128:(i + 1) * 128],
                                                 h_prev_ap(j, n), start=(j == 0), stop=False)
                            nc.tensor.matmul(pz[:], ident[:], At["z"][i][:],
                                             start=False, stop=True)
                            a_t = abpool.tile([128, NCOL], f16, tag="a")
                            nc.scalar.activation(a_t[:], pz[:], AF.Sigmoid, scale=-1.0)
                            a_list.append(a_t)
                            # reset gate
                            pr = pspool.tile([128, NCOL], f32, tag="ps")
                            for j in range(4):
                                nc.tensor.matmul(pr[:], u_t["r"][j][:, i * 128:(i + 1) * 128],
                                                 h_prev_ap(j, n), start=(j == 0), stop=False)
                            nc.tensor.matmul(pr[:], ident[:], At["r"][i][:],
                                             start=False, stop=True)
                            r_t = tpool.tile([128, NCOL], f16, tag="rt")
                            nc.scalar.activation(r_t[:], pr[:], AF.Sigmoid)
                            rh = tpool.tile([128, NCOL], f16, tag="rh")
                            nc.vector.tensor_mul(
                                rh[:].rearrange("p (b t) -> p b t", b=B_CORE),
                                r_t[:].rearrange("p (b t) -> p b t", b=B_CORE),
                                h_prev_ap(i, n))
                            rh_list.append(rh)
                        bneg_list = []
                        for i in range(4):
                            ph = pspool.tile([128, NCOL], f32, tag="ps")
                            for j in range(4):
                                nc.tensor.matmul(ph[:], u_t["h"][j][:, i * 128:(i + 1) * 128],
                                                 rh_list[j][:], start=(j == 0), stop=False)
                            nc.tensor.matmul(ph[:], ident[:], At["h"][i][:],
                                             start=False, stop=True)
                            ht_t = tpool.tile([128, NCOL], f16, tag="ht")
                            nc.scalar.activation(ht_t[:], ph[:], AF.Tanh)
                            # bneg = (a-1)*h_tilde = -z*h_tilde
                            bn = bbpool.tile([128, NCOL], f16, tag="bn")
                            nc.vector.scalar_tensor_tensor(
                                bn[:], a_list[i][:], 1.0, ht_t[:],
                                OP.subtract, OP.mult)
                            bneg_list.append(bn)
                        scan_q.append((n, a_list, bneg_list))
                        # lagged scans: emit scans for subchunk n-1
                        if len(scan_q) >= 2:
                            _emit_scan(nc, OP, hT, scan_q.pop(0), B_CORE, SUB, KW, NSUB)
                    while scan_q:
                        _emit_scan(nc, OP, hT, scan_q.pop(0), B_CORE, SUB, KW, NSUB)

                # ---- phase 3: outs for this chunk ----
                for n in range(NSUB):
                    t0 = c0 + n * SUB
                    for m in range(2):
                        po = pspool.tile([128, NCOL], f32, tag="ps")
                        for j in range(4):
                            nc.tensor.matmul(po[:], wb_t[j][:, m * 128:(m + 1) * 128],
                                             h_cur_ap(j, n), start=(j == 0), stop=(j == 3))
                        oe = oevpool.tile([128, NCOL], f32, tag="oe")
                        nc.scalar.activation(oe[:], po[:], AF.Identity,
                                             bias=bb_t[:, m:m + 1])
                        dst = outs_d[m * 128:(m + 1) * 128, :] \
                            .rearrange("p (b t) -> p b t", b=B_CORE)[:, :, t0:t0 + SUB]
                        nc.gpsimd.dma_start(
                            dst, oe[:].rearrange("p (b t) -> p b t", b=B_CORE))

                # ---- carry boundary to next chunk ----
                for j in range(4):
                    nc.vector.tensor_copy(hb[j][:], hT[j][NSUB - 1][:, SUB::KW])

            # ---- final hidden state ----
            hf = oevpool.tile([128, 4 * B_CORE], f32, tag="hf")
            for j in range(4):
                nc.vector.tensor_copy(hf[:, j * B_CORE:(j + 1) * B_CORE], hb[j][:])
            nc.sync.dma_start(hfin_d[:, :], hf[:])

    nc.compile()
    return nc


def _emit_scan(nc, OP, hT, item, B, SUB, KW, NSUB):
    n, a_list, bneg_list = item
    for j in range(4):
        ht = hT[j][n][:]
        at = a_list[j][:]
        bt = bneg_list[j][:]
        for b in range(B):
            # state = a*state - bneg  (bneg = -z*h_tilde)
            nc.vector.tensor_tensor_scan(
                ht[:, b * KW + 1:(b + 1) * KW],
                at[:, b * SUB:(b + 1) * SUB],
                bt[:, b * SUB:(b + 1) * SUB],
                ht[:, b * KW:b * KW + 1], OP.mult, OP.subtract)
        # propagate boundary into next subchunk tile (col SUB -> col 0)
        if n + 1 < NSUB:
            nc.vector.tensor_copy(hT[j][n + 1][:, 0::KW], hT[j][n][:, SUB::KW])


def _get_nc(T, sweeps):
    key = (T, sweeps)
    if key not in _CACHE:
        _CACHE[key] = _build(T, sweeps)
    return _CACHE[key]


def kernel(x, Wz, bz, Wr, br, Wh, bh, Wb, bb):
    from concourse.bass_utils import run_bass_kernel_spmd

    x = np.asarray(x)
    B_full, _, T, I = x.shape
    sweeps = int(os.environ.get("DEER_SWEEPS", "6"))
    nc = _get_nc(T, sweeps)

    f16 = np.float16
    Wz, Wr, Wh, Wb = (np.asarray(a, np.float32) for a in (Wz, Wr, Wh, Wb))
    bz, br, bh, bb = (np.asarray(a, np.float32) for a in (bz, br, bh, bb))

    common = {}
    for g, W in (("z", Wz), ("r", Wr), ("h", Wh)):
        common[f"wx{g}T"] = np.ascontiguousarray(W[:, :I3].T, f16)
        common[f"u{g}T"] = np.ascontiguousarray(W[:, I3:].T, f16)
    common["wbT"] = np.ascontiguousarray(Wb.T, f16)
    for g, b in (("z", bz), ("r", br), ("h", bh)):
        common[f"b{g}"] = np.ascontiguousarray(b.reshape(4, 128).T, np.float32)
    common["bb"] = np.ascontiguousarray(bb.reshape(2, 128).T, np.float32)
    common["ident"] = np.eye(128, dtype=f16)

    in_maps = []
    for c in range(N_CORES):
        xc = x[c * B_CORE:(c + 1) * B_CORE]          # [8, 3, T, I]
        # xsT [768, B*T] with col = b*T + t ; row = s*I + i
        xst = np.ascontiguousarray(
            xc.transpose(1, 3, 0, 2).reshape(I3, B_CORE * T), f16)
        m = dict(common)
        m["xsT"] = xst
        in_maps.append(m)

    res = run_bass_kernel_spmd(nc, in_maps, core_ids=list(range(N_CORES)))

    outs = np.empty((B_full, T, BD), np.float32)
    hfin = np.empty((B_full, H), np.float32)
    for c in range(N_CORES):
        o = res.results[c]["outs"].reshape(BD, B_CORE, T)
        outs[c * B_CORE:(c + 1) * B_CORE] = o.transpose(1, 2, 0)
        hf = res.results[c]["hfin"].reshape(128, 4, B_CORE)
        hfin[c * B_CORE:(c + 1) * B_CORE] = hf.transpose(2, 1, 0).reshape(B_CORE, H)
    return outs, hfin


# revision 8
# speedup vs baseline: 1.5679x; 1.5539x over previous
"""Trainium2 Bass kernel for a GRU-style recurrent cell.

Strategy
--------
8-way data parallel over the batch (8 rows/core, weights replicated).
Per core, the sequential scan is replaced by a quasi-DEER fixed-point
iteration: the diagonal gating recurrence h_t = a_t*h_{t-1} + b_t is
solved exactly each sweep with the DVE tensor_tensor_scan, while the
dense U-matmul couplings are iterated (Jacobi/Gauss-Seidel hybrid).
Empirically this converges to ~3e-5 relative error in 7 sweeps per
chunk, chunk-length independent, so the whole recurrence costs ~7
batched GEMM passes instead of 2048 tiny sequential matmuls (which are
LDWEIGHTS-bound on the PE).

Phases per 512-step chunk:
  1) precompute A_g = W_gx @ x + b_g for the 3 gates (fp16, spilled to DRAM)
  2) SWEEPS fixed-point sweeps; each sweep streams A back, does the
     3 U-GEMMs + activations + the chained scans per 64-step subchunk
  3) outs = Wb @ h + bb for the converged chunk trajectory
"""

import os
import numpy as np

T_FULL = 2048
B_CORE = 8          # batch rows per core
H = 512             # hidden dim
I3 = 768            # 3*I_DIM, x-part of the cell input
BD = 256            # output dim of Wb
N_CORES = 8
SUB = 64            # timesteps per subchunk (512 psum cols = 64*8)
NCOL = SUB * B_CORE # 512 columns per matmul tile

_CACHE = {}


def _build(T, sweeps):
    import concourse.bass as bass
    import concourse.tile as tile
    from concourse import bacc, mybir

    f16 = mybir.dt.float16
    f32 = mybir.dt.float32
    AF = mybir.ActivationFunctionType
    OP = mybir.AluOpType

    L = min(512, T)          # chunk length
    NCH = T // L             # chunks
    NSUB = L // SUB          # subchunks per chunk

    nc = bacc.Bacc("TRN2", target_bir_lowering=False, debug=False,
                   num_devices=N_CORES)

    # ---- DRAM I/O (per core) ----
    xsT = nc.dram_tensor("xsT", [I3, B_CORE * T], f16, kind="ExternalInput").ap()
    wxT = {g: nc.dram_tensor(f"wx{g}T", [I3, H], f16, kind="ExternalInput").ap()
           for g in "zrh"}
    uT = {g: nc.dram_tensor(f"u{g}T", [H, H], f16, kind="ExternalInput").ap()
          for g in "zrh"}
    wbT = nc.dram_tensor("wbT", [H, BD], f16, kind="ExternalInput").ap()
    bias = {g: nc.dram_tensor(f"b{g}", [128, 4], f32, kind="ExternalInput").ap()
            for g in "zrh"}
    bbv = nc.dram_tensor("bb", [128, 2], f32, kind="ExternalInput").ap()
    ident_d = nc.dram_tensor("ident", [128, 128], f16, kind="ExternalInput").ap()

    outs_d = nc.dram_tensor("outs", [BD, B_CORE * T], f32, kind="ExternalOutput").ap()
    hfin_d = nc.dram_tensor("hfin", [128, 4 * B_CORE], f32, kind="ExternalOutput").ap()

    with tile.TileContext(nc) as tc:
        with (
            tc.tile_pool(name="dram", bufs=1, space="DRAM") as dpool,
            tc.tile_pool(name="wpool", bufs=1) as wpool,
            tc.tile_pool(name="hpool", bufs=1) as hpool,
            tc.tile_pool(name="xs", bufs=12) as xspool,
            tc.tile_pool(name="aload", bufs=26) as apool,
            tc.tile_pool(name="ev", bufs=6) as evpool,
            tc.tile_pool(name="abuf", bufs=10) as abpool,
            tc.tile_pool(name="bbuf", bufs=10) as bbpool,
            tc.tile_pool(name="tmp", bufs=6) as tpool,
            tc.tile_pool(name="oev", bufs=3) as oevpool,
            tc.tile_pool(name="psum", bufs=8, space="PSUM") as pspool,
        ):
            # A_g spill tensors in DRAM, [512, B*T] cols = b*T + t
            A_d = {g: dpool.tile([H, B_CORE * T], f16, tag=f"A{g}", name=f"A{g}") for g in "zrh"}

            # ---- load constant weights into SBUF ----
            wx_t = {}   # per gate: 6 tiles [128, 512] (k-tile rows)
            u_t = {}    # per gate: 4 tiles [128, 512]
            for g in "zrh":
                wx_t[g] = []
                for j in range(6):
                    w = wpool.tile([128, H], f16, tag=f"wx{g}{j}")
                    nc.sync.dma_start(w[:], wxT[g][j * 128:(j + 1) * 128, :])
                    wx_t[g].append(w)
                u_t[g] = []
                for j in range(4):
                    w = wpool.tile([128, H], f16, tag=f"u{g}{j}")
                    nc.sync.dma_start(w[:], uT[g][j * 128:(j + 1) * 128, :])
                    u_t[g].append(w)
            wb_t = []
            for j in range(4):
                w = wpool.tile([128, BD], f16, tag=f"wb{j}")
                nc.sync.dma_start(w[:], wbT[j * 128:(j + 1) * 128, :])
                wb_t.append(w)
            ident = wpool.tile([128, 128], f16, tag="ident")
            nc.sync.dma_start(ident[:], ident_d[:, :])
            bias_t = {}
            for g in "zrh":
                bt = wpool.tile([128, 4], f32, tag=f"bias{g}")
                nc.sync.dma_start(bt[:], bias[g][:, :])
                bias_t[g] = bt
            bb_t = wpool.tile([128, 2], f32, tag="biasb")
            nc.sync.dma_start(bb_t[:], bbv[:, :])

            # persistent h trajectory tiles: hT[j][n] = [128, 8*(SUB+1)]
            # col b*(SUB+1) + k ; k=0 boundary (h at step n*SUB-1), k=1..SUB
            # h at steps n*SUB .. n*SUB+SUB-1 (local to chunk)
            KW = SUB + 1
            hT = [[hpool.tile([128, B_CORE * KW], f16, tag=f"h{j}_{n}", name=f"h{j}_{n}")
                   for n in range(NSUB)] for j in range(4)]
            hb = [hpool.tile([128, B_CORE], f16, tag=f"hb{j}", name=f"hb{j}") for j in range(4)]
            for j in range(4):
                nc.vector.memset(hb[j][:], 0.0)

            def h_prev_ap(j, n):
                # [128, 8, SUB] free dims (b, t): h_{t-1} for steps in subchunk n
                return hT[j][n][:].rearrange("p (b k) -> p b k", b=B_CORE)[:, :, 0:SUB]

            def h_cur_ap(j, n):
                return hT[j][n][:].rearrange("p (b k) -> p b k", b=B_CORE)[:, :, 1:KW]

            for c in range(NCH):
                c0 = c * L

                # ---- phase 1: A_g for this chunk ----
                for n in range(NSUB):
                    t0 = c0 + n * SUB
                    xs_tiles = []
                    for j in range(6):
                        xt = xspool.tile([128, NCOL], f16, tag="xs")
                        src = xsT[j * 128:(j + 1) * 128, :] \
                            .rearrange("p (b t) -> p b t", b=B_CORE)[:, :, t0:t0 + SUB]
                        nc.sync.dma_start(xt[:].rearrange("p (b t) -> p b t", b=B_CORE), src)
                        xs_tiles.append(xt)
                    for g in "zrh":
                        for i in range(4):
                            ps = pspool.tile([128, NCOL], f32, tag="ps")
                            for j in range(6):
                                nc.tensor.matmul(
                                    ps[:], wx_t[g][j][:, i * 128:(i + 1) * 128],
                                    xs_tiles[j][:], start=(j == 0), stop=(j == 5))
                            ev = evpool.tile([128, NCOL], f16, tag="ev")
                            nc.scalar.activation(ev[:], ps[:], AF.Identity,
                                                 bias=bias_t[g][:, i:i + 1])
                            dst = A_d[g][i * 128:(i + 1) * 128, :] \
                                .rearrange("p (b t) -> p b t", b=B_CORE)[:, :, t0:t0 + SUB]
                            nc.gpsimd.dma_start(
                                dst, ev[:].rearrange("p (b t) -> p b t", b=B_CORE))

                # ---- init h trajectory to zero + boundary column ----
                # (zero init makes sweep 0 exact with U.h = 0 and r*h = 0,
                # so sweep 0 needs no matmuls at all)
                for j in range(4):
                    for n in range(NSUB):
                        nc.vector.memset(hT[j][n][:], 0.0)
                    if c > 0:
                        nc.vector.tensor_copy(hT[j][0][:, 0::KW], hb[j][:])

                # ---- phase 2: sweeps ----
                # sweep 0 runs on the zero trajectory: U.h and r*h vanish, so
                # a = sigma(-Az), h_tilde = tanh(Ah) come straight from the
                # A tiles with no matmuls and no reset gate at all.
                for s in range(sweeps):
                    cheap = (s == 0)
                    scan_q = []
                    for n in range(NSUB):
                        t0 = c0 + n * SUB
                        # A tiles for this subchunk
                        At = {}
                        for g in ("zh" if cheap else "zrh"):
                            At[g] = []
                            for i in range(4):
                                at = apool.tile([128, NCOL], f16, tag="at")
                                src = A_d[g][i * 128:(i + 1) * 128, :] \
                                    .rearrange("p (b t) -> p b t", b=B_CORE)[:, :, t0:t0 + SUB]
                                nc.sync.dma_start(
                                    at[:].rearrange("p (b t) -> p b t", b=B_CORE), src)
                                At[g].append(at)
                        if cheap:
                            a_list, bneg_list = [], []
                            for i in range(4):
                                a_t = abpool.tile([128, NCOL], f16, tag="a")
                                nc.scalar.activation(a_t[:], At["z"][i][:],
                                                     AF.Sigmoid, scale=-1.0)
                                a_list.append(a_t)
                                ht_t = tpool.tile([128, NCOL], f16, tag="ht")
                                nc.scalar.activation(ht_t[:], At["h"][i][:], AF.Tanh)
                                bn = bbpool.tile([128, NCOL], f16, tag="bn")
                                nc.vector.scalar_tensor_tensor(
                                    bn[:], a_list[i][:], 1.0, ht_t[:],
                                    OP.subtract, OP.mult)
                                bneg_list.append(bn)
                            scan_q.append((n, a_list, bneg_list))
                            if len(scan_q) >= 2:
                                _emit_scan(nc, OP, hT, scan_q.pop(0), B_CORE, SUB, KW, NSUB)
                            continue
                        a_list, rh_list = [], []
                        for i in range(4):
                            # update gate -> a = sigma(-pre_z) = 1 - z
                            pz = pspool.tile([128, NCOL], f32, tag="ps")
                            for j in range(4):
                                nc.tensor.matmul(pz[:], u_t["z"][j][:, i * 128:(i + 1) * 128],
                                                 h_prev_ap(j, n), start=(j == 0), stop=False)
                            nc.tensor.matmul(pz[:], ident[:], At["z"][i][:],
                                             start=False, stop=True)
                            a_t = abpool.tile([128, NCOL], f16, tag="a")
                            nc.scalar.activation(a_t[:], pz[:], AF.Sigmoid, scale=-1.0)
                            a_list.append(a_t)
                            # reset gate
                            pr = pspool.tile([128, NCOL], f32, tag="ps")
                            for j in range(4):
                                nc.tensor.matmul(pr[:], u_t["r"][j][:, i * 128:(i + 1) * 128],
                                                 h_prev_ap(j, n), start=(j == 0), stop=False)
                            nc.tensor.matmul(pr[:], ident[:], At["r"][i][:],
                                             start=False, stop=True)
                            r_t = tpool.tile([128, NCOL], f16, tag="rt")
                            nc.scalar.activation(r_t[:], pr[:], AF.Sigmoid)
                            rh = tpool.tile([128, NCOL], f16, tag="rh")
                            nc.vector.tensor_mul(
                                rh[:].rearrange("p (b t) -> p b t", b=B_CORE),
                                r_t[:].rearrange("p (b t) -> p b t", b=B_CORE),
                                h_prev_ap(i, n))
                            rh_list.append(rh)
                        bneg_list = []
                        for i in range(4):
                            ph = pspool.tile([128, NCOL], f32, tag="ps")
                            for j in range(4):
                                nc.tensor.matmul(ph[:], u_t["h"][j][:, i * 128:(i + 1) * 128],
                                                 rh_list[j][:], start=(j == 0), stop=False)
                            nc.tensor.matmul(ph[:], ident[:], At["h"][i][:],
                                             start=False, stop=True)
                            ht_t = tpool.tile([128, NCOL], f16, tag="ht")
                            nc.scalar.activation(ht_t[:], ph[:], AF.Tanh)
                            # bneg = (a-1)*h_tilde = -z*h_tilde
                            bn = bbpool.tile([128, NCOL], f16, tag="bn")
                            nc.vector.scalar_tensor_tensor(
                                bn[:], a_list[i][:], 1.0, ht_t[:],
                                OP.subtract, OP.mult)
                            bneg_list.append(bn)
                        scan_q.append((n, a_list, bneg_list))
                        # lagged scans: emit scans for subchunk n-1
                        if len(scan_q) >= 2:
                            _emit_scan(nc, OP, hT, scan_q.pop(0), B_CORE, SUB, KW, NSUB)
                    while scan_q:
                        _emit_scan(nc, OP, hT, scan_q.pop(0), B_CORE, SUB, KW, NSUB)

                # ---- phase 3: outs for this chunk ----
                for n in range(NSUB):
                    t0 = c0 + n * SUB
                    for m in range(2):
                        po = pspool.tile([128, NCOL], f32, tag="ps")
                        for j in range(4):
                            nc.tensor.matmul(po[:], wb_t[j][:, m * 128:(m + 1) * 128],
                                             h_cur_ap(j, n), start=(j == 0), stop=(j == 3))
                        oe = oevpool.tile([128, NCOL], f32, tag="oe")
                        nc.scalar.activation(oe[:], po[:], AF.Identity,
                                             bias=bb_t[:, m:m + 1])
                        dst = outs_d[m * 128:(m + 1) * 128, :] \
                            .rearrange("p (b t) -> p b t", b=B_CORE)[:, :, t0:t0 + SUB]
                        nc.gpsimd.dma_start(
                            dst, oe[:].rearrange("p (b t) -> p b t", b=B_CORE))

                # ---- carry boundary to next chunk ----
                for j in range(4):
                    nc.vector.tensor_copy(hb[j][:], hT[j][NSUB - 1][:, SUB::KW])

            # ---- final hidden state ----
            hf = oevpool.tile([128, 4 * B_CORE], f32, tag="hf")
            for j in range(4):
                nc.vector.tensor_copy(hf[:, j * B_CORE:(j + 1) * B_CORE], hb[j][:])
            nc.sync.dma_start(hfin_d[:, :], hf[:])

    nc.compile()
    return nc


def _emit_scan(nc, OP, hT, item, B, SUB, KW, NSUB):
    n, a_list, bneg_list = item
    for j in range(4):
        ht = hT[j][n][:]
        at = a_list[j][:]
        bt = bneg_list[j][:]
        for b in range(B):
            # state = a*state - bneg  (bneg = -z*h_tilde)
            nc.vector.tensor_tensor_scan(
                ht[:, b * KW + 1:(b + 1) * KW],
                at[:, b * SUB:(b + 1) * SUB],
                bt[:, b * SUB:(b + 1) * SUB],
                ht[:, b * KW:b * KW + 1], OP.mult, OP.subtract)
        # propagate boundary into next subchunk tile (col SUB -> col 0)
        if n + 1 < NSUB:
            nc.vector.tensor_copy(hT[j][n + 1][:, 0::KW], hT[j][n][:, SUB::KW])


def _get_nc(T, sweeps):
    key = (T, sweeps)
    if key not in _CACHE:
        _CACHE[key] = _build(T, sweeps)
    return _CACHE[key]


def kernel(x, Wz, bz, Wr, br, Wh, bh, Wb, bb):
    from concourse.bass_utils import run_bass_kernel_spmd

    x = np.asarray(x)
    B_full, _, T, I = x.shape
    sweeps = int(os.environ.get("DEER_SWEEPS", "5"))
    nc = _get_nc(T, sweeps)

    f16 = np.float16
    Wz, Wr, Wh, Wb = (np.asarray(a, np.float32) for a in (Wz, Wr, Wh, Wb))
    bz, br, bh, bb = (np.asarray(a, np.float32) for a in (bz, br, bh, bb))

    common = {}
    for g, W in (("z", Wz), ("r", Wr), ("h", Wh)):
        common[f"wx{g}T"] = np.ascontiguousarray(W[:, :I3].T, f16)
        common[f"u{g}T"] = np.ascontiguousarray(W[:, I3:].T, f16)
    common["wbT"] = np.ascontiguousarray(Wb.T, f16)
    for g, b in (("z", bz), ("r", br), ("h", bh)):
        common[f"b{g}"] = np.ascontiguousarray(b.reshape(4, 128).T, np.float32)
    common["bb"] = np.ascontiguousarray(bb.reshape(2, 128).T, np.float32)
    common["ident"] = np.eye(128, dtype=f16)

    in_maps = []
    for c in range(N_CORES):
        xc = x[c * B_CORE:(c + 1) * B_CORE]          # [8, 3, T, I]
        # xsT [768, B*T] with col = b*T + t ; row = s*I + i
        xst = np.ascontiguousarray(
            xc.transpose(1, 3, 0, 2).reshape(I3, B_CORE * T), f16)
        m = dict(common)
        m["xsT"] = xst
        in_maps.append(m)

    res = run_bass_kernel_spmd(nc, in_maps, core_ids=list(range(N_CORES)))

    outs = np.empty((B_full, T, BD), np.float32)
    hfin = np.empty((B_full, H), np.float32)
    for c in range(N_CORES):
        o = res.results[c]["outs"].reshape(BD, B_CORE, T)
        outs[c * B_CORE:(c + 1) * B_CORE] = o.transpose(1, 2, 0)
        hf = res.results[c]["hfin"].reshape(128, 4, B_CORE)
        hfin[c * B_CORE:(c + 1) * B_CORE] = hf.transpose(2, 1, 0).reshape(B_CORE, H)
    return outs, hfin
